# revision 9
# baseline (speedup 1.0000x reference)
"""AdaAttN attention kernel for 8 TRN2 NeuronCores (v2).

Problem: nn_AdaAttN_29076928593982
  fc, fs, fcs: (4, 4096, 256) f32; Wf/Wg/Wh (256,256); bf/bg/bh (256,)
  Q = Wf@inorm(fc_t)+bf; K = Wg@inorm(fs_t)+bg; V = Wh@fs_t+bh
  A = softmax(Q K); M = A V; Var = A V^2 - M^2; S = sqrt(max(Var,1e-6))
  out = S * inorm(fcs_t) + M   (all in (b, t, d))

Sharding: data-parallel over (sample, query-half): core i -> sample i//2,
query rows [ (i%2)*2048, +2048 ). K/V replicated per sample. No collectives.

v2 changes over the phase-serial baseline (302us):
  - V projection needs NO instance norm, so it streams chunk-by-chunk
    against the fs DMA: the PE starts working ~2us in and never idles
    long enough for the HAM clock-gate to re-throttle.
  - One fused emission order (PE executes its queue in order): V proj ->
    K bias/proj -> Q bias/proj -> attention; fc/fcs stats run on ACT/DVE
    interleaved so no engine stream blocks on a DMA that lands later.
  - softmax tile (st), V and V^2 are bf16: bf16 stationaries get FWL
    (~4x faster LDWEIGHTS), un-saturating the PE weight path that f32r
    (no FWL) saturates; A-quantization at 0.4%/weight is benign since
    per-row scale errors cancel in M = (A@V)/(A@1).
  - sqrt via Exp(0.5*Ln(x)): Exp and Sqrt never share an ACT function
    table (22 table loads = 28us in the baseline trace); Ln+Exp co-reside
    in natural_log_exp_and_others so the table loads once.
  - logits stay f32r (bf16 Q/K would put ~0.07 abs noise on logits which
    the softmax exponentiates to ~7% weight errors).
"""
import sys

sys.path.insert(0, "/opt/trn_rl_repo")

import numpy as np

import concourse.bass as bass
import concourse.tile as tile
from concourse import bacc
from concourse import mybir
from concourse.bass_utils import run_bass_kernel_spmd

F32 = mybir.dt.float32
F32R = mybir.dt.float32r
BF16 = mybir.dt.bfloat16
AF = mybir.ActivationFunctionType
OP = mybir.AluOpType

P = 128          # partitions
D = 256          # feature dim
T = 4096         # tokens per sample
TH = 2048        # query tokens per core
CH = 2           # channel chunks (D // P)
NB = T // P      # tk chunks (32)
NQ = TH // 256   # tq chunks of 256 (8)
C0 = 110.0       # global softmax shift
EPS_IN = 1e-5
EPS_VAR = 1e-6
CK = 1024        # stats DMA chunk width
NCK = T // CK

DT_AV = BF16     # dtype for softmax tile / V / V^2 (accuracy fallback: F32R)

TRACE = False    # test.py sets this to get exec_time_ns
TRACE_KW = {}


def _bcast_row(handle, offset, n):
    """AP reading a DRAM row of n elements broadcast across 128 partitions."""
    return bass.AP(tensor=handle, offset=offset, ap=[[0, P], [1, n]])


def build_nc():
    nc = bacc.Bacc()

    fcT = nc.declare_dram_parameter("fcT", [D, T], F32, isOutput=False)
    fsT = nc.declare_dram_parameter("fsT", [D, T], F32, isOutput=False)
    fcsT = nc.declare_dram_parameter("fcsT", [D, T], F32, isOutput=False)
    fcsh = nc.declare_dram_parameter("fcsh", [TH, D], F32, isOutput=False)
    wfT = nc.declare_dram_parameter("wfT", [D, D], F32, isOutput=False)
    wgT = nc.declare_dram_parameter("wgT", [D, D], F32, isOutput=False)
    whT = nc.declare_dram_parameter("whT", [D, D], F32, isOutput=False)
    bq_e = nc.declare_dram_parameter("bq", [D, 1], F32, isOutput=False)
    bk_e = nc.declare_dram_parameter("bk", [D, 1], F32, isOutput=False)
    bv_e = nc.declare_dram_parameter("bv", [D], F32, isOutput=False)
    out_e = nc.declare_dram_parameter("out", [TH, D], F32, isOutput=True)

    scm = nc.dram_tensor("scm", [2, D], F32)  # fcs stats roundtrip scratch

    with tile.TileContext(nc) as tc:
        persist_cm = tc.tile_pool(name="persist", bufs=1)
        pp = persist_cm.__enter__()

        QTr = [pp.tile([P, TH], F32R, name=f"qtr{c}", tag=f"qtr{c}") for c in range(CH)]
        KTr = [pp.tile([P, T], F32R, name=f"ktr{c}", tag=f"ktr{c}") for c in range(CH)]
        Vr = pp.tile([P, NB, D + 2], DT_AV, name="vr", tag="vr")  # [V | ones | pad]
        V2r = pp.tile([P, NB, D], DT_AV, name="v2r", tag="v2r")
        nfcs = pp.tile([P, TH // P, D], F32, name="nfcs", tag="nfcs")
        bqe = [pp.tile([P, 1], F32, name=f"bqe{c}", tag=f"bqe{c}") for c in range(CH)]
        bke = [pp.tile([P, 1], F32, name=f"bke{c}", tag=f"bke{c}") for c in range(CH)]
        bv_bc = pp.tile([P, D], F32, name="bvbc", tag="bvbc")
        m_bc = pp.tile([P, D], F32, name="mbc", tag="mbc")
        i_bc = pp.tile([P, D], F32, name="ibc", tag="ibc")
        eps_t = pp.tile([P, 1], F32, name="epsin", tag="epsin")
        negc0_t = pp.tile([P, 1], F32, name="negc0", tag="negc0")

        # weight staging + folded copies, live for the whole prologue
        pw_cm = tc.tile_pool(name="pw", bufs=1)
        pw = pw_cm.__enter__()
        wf_sb = [pw.tile([P, D], F32, name=f"wf{c}", tag=f"wf{c}") for c in range(CH)]
        wg_sb = [pw.tile([P, D], F32, name=f"wg{c}", tag=f"wg{c}") for c in range(CH)]
        wh_sb = [pw.tile([P, D], F32, name=f"wh{c}", tag=f"wh{c}") for c in range(CH)]
        bq_sb = [pw.tile([P, 1], F32, name=f"bqs{c}", tag=f"bqs{c}") for c in range(CH)]
        bk_sb = [pw.tile([P, 1], F32, name=f"bks{c}", tag=f"bks{c}") for c in range(CH)]
        wq = [pw.tile([P, D], F32R, name=f"wq{c}", tag=f"wq{c}") for c in range(CH)]
        wk = [pw.tile([P, D], F32R, name=f"wk{c}", tag=f"wk{c}") for c in range(CH)]
        wv = [pw.tile([P, D], F32R, name=f"wv{c}", tag=f"wv{c}") for c in range(CH)]

        # weight DMAs first in queue order, then fs, fc, fcs
        for c in range(CH):
            nc.sync.dma_start(out=wh_sb[c], in_=whT[c * P : (c + 1) * P, :])
            nc.sync.dma_start(out=wg_sb[c], in_=wgT[c * P : (c + 1) * P, :])
            nc.sync.dma_start(out=wf_sb[c], in_=wfT[c * P : (c + 1) * P, :])
            nc.sync.dma_start(out=bq_sb[c], in_=bq_e[c * P : (c + 1) * P, :])
            nc.sync.dma_start(out=bk_sb[c], in_=bk_e[c * P : (c + 1) * P, :])
        nc.gpsimd.dma_start(out=bv_bc, in_=_bcast_row(bv_e, 0, D))

        nc.vector.memset(eps_t, EPS_IN)
        nc.vector.memset(negc0_t, -C0)
        ones_f32 = pw.tile([P, NB * 2], F32, name="ones32", tag="ones32")
        nc.vector.memset(ones_f32, 1.0)
        nc.vector.tensor_copy(
            Vr[:, :, D : D + 2], ones_f32.rearrange("p (n two) -> p n two", two=2)
        )
        for c in range(CH):
            nc.vector.tensor_copy(wv[c], wh_sb[c])

        def stats_chunk(x_ext, name, ring, k, c, acc_s, acc_q, round_to=None, round_cols=0):
            """Load chunk (c,k) of a (D,T) DRAM tensor, accumulate sum and
            sumsq (ACT/DVE alternating), writing the f32r rounded copy."""
            ck = ring.tile([P, CK], F32, name=f"{name}ck{c}_{k}", tag=f"{name}ck", bufs=4)
            nc.sync.dma_start(
                out=ck, in_=x_ext[c * P : (c + 1) * P, k * CK : (k + 1) * CK]
            )
            scr = pw.tile([P, CK], F32, name=f"{name}scr", tag="scr", bufs=1)
            if round_to is not None and (k + 1) * CK <= round_cols:
                dst = round_to[c][:, k * CK : (k + 1) * CK]
            else:
                dst = scr
            scr2 = pw.tile([P, CK], F32, name=f"{name}scr2", tag="scr2", bufs=1)
            if (2 * k + c) % 2 == 0:
                nc.scalar.activation(dst, ck, AF.Copy, accum_out=acc_s[c][:, k : k + 1])
                nc.vector.scalar_tensor_tensor(
                    scr2, ck, 0.0, ck, op0=OP.add, op1=OP.mult,
                    accum_out=acc_q[c][:, k : k + 1],
                )
            else:
                nc.vector.tensor_scalar(
                    dst, ck, 0.0, 0.0, op0=OP.add, op1=OP.add,
                    accum_out=acc_s[c][:, k : k + 1],
                )
                nc.scalar.activation(
                    scr2, ck, AF.Square, accum_out=acc_q[c][:, k : k + 1]
                )

        # ---------------- fs phase: V proj streamed against the DMA -------
        pfs_cm = tc.tile_pool(name="pfs", bufs=1)
        pfs = pfs_cm.__enter__()
        fsr = [pfs.tile([P, T], F32R, name=f"fsr{c}", tag=f"fsr{c}") for c in range(CH)]
        acc_s_fs = [pfs.tile([P, NCK], F32, name=f"fsas{c}", tag=f"fsas{c}") for c in range(CH)]
        acc_q_fs = [pfs.tile([P, NCK], F32, name=f"fsaq{c}", tag=f"fsaq{c}") for c in range(CH)]

        psv_cm = tc.tile_pool(name="psv", bufs=3, space="PSUM")
        psv = psv_cm.__enter__()

        for k in range(NCK):
            for c in range(CH):
                stats_chunk(fsT, "fs", pfs, k, c, acc_s_fs, acc_q_fs,
                            round_to=fsr, round_cols=T)
            # V proj for the 8 token blocks this chunk completes
            # (V = Wh @ fs + bh has no instance norm: no stats dependency)
            for tb in range(8 * k, 8 * (k + 1)):
                pv = psv.tile([P, D], F32, name=f"pv{tb}", tag="pv")
                sl = slice(tb * P, (tb + 1) * P)
                nc.tensor.matmul(pv, fsr[0][:, sl], wv[0], start=True, stop=False)
                nc.tensor.matmul(pv, fsr[1][:, sl], wv[1], start=False, stop=True)
                if tb % 2 == 0:
                    nc.scalar.activation(Vr[:, tb, 0:D], pv, AF.Copy)
                else:
                    nc.vector.tensor_copy(Vr[:, tb, 0:D], pv)
                nc.vector.tensor_mul(V2r[:, tb, :], Vr[:, tb, 0:D], Vr[:, tb, 0:D])

        psv_cm.__exit__(None, None, None)

        # fs stats: mean + inv_std (rsqrt via Ln/Exp, no table swap)
        m_s, i_s = [], []
        for c in range(CH):
            m = pfs.tile([P, 1], F32, name=f"fsm{c}", tag=f"fsm{c}")
            nc.vector.reduce_sum(m, acc_s_fs[c], axis=mybir.AxisListType.X)
            nc.vector.tensor_scalar_mul(m, m, 1.0 / T)
            v = pfs.tile([P, 1], F32, name=f"fsv{c}", tag=f"fsv{c}")
            nc.vector.reduce_sum(v, acc_q_fs[c], axis=mybir.AxisListType.X)
            nc.vector.tensor_scalar_mul(v, v, 1.0 / T)
            msq = pfs.tile([P, 1], F32, name=f"fsmsq{c}", tag=f"fsmsq{c}")
            nc.vector.tensor_mul(msq, m, m)
            nc.vector.tensor_sub(v, v, msq)
            nc.scalar.activation(v, v, AF.Ln, bias=eps_t)
            nc.scalar.activation(v, v, AF.Exp, scale=-0.5)
            m_s.append(m)
            i_s.append(v)

        psb_cm = tc.tile_pool(name="psb", bufs=2, space="PSUM")
        psb = psb_cm.__enter__()

        for c in range(CH):
            nc.vector.tensor_scalar_mul(wk[c], wg_sb[c], i_s[c])
        m_sr = [pfs.tile([P, 2], F32R, name=f"fsmr{c}", tag=f"fsmr{c}") for c in range(CH)]
        for c in range(CH):
            nc.vector.tensor_copy(m_sr[c], m_s[c].to_broadcast((P, 2)))
        for oc in range(CH):
            pb = psb.tile([P, 2], F32, name=f"pbk{oc}", tag="pbk")
            nc.tensor.matmul(pb, wk[0][:, oc * P : (oc + 1) * P], m_sr[0], start=True, stop=False)
            nc.tensor.matmul(pb, wk[1][:, oc * P : (oc + 1) * P], m_sr[1], start=False, stop=True)
            nc.vector.tensor_sub(bke[oc], bk_sb[oc], pb[:, 0:1])

        # ---------------- fc stats stream + K^T projection interleaved ----
        pfc_cm = tc.tile_pool(name="pfc", bufs=1)
        pfc = pfc_cm.__enter__()
        fcr = [pfc.tile([P, TH], F32R, name=f"fcr{c}", tag=f"fcr{c}") for c in range(CH)]
        acc_s_fc = [pfc.tile([P, NCK], F32, name=f"fcas{c}", tag=f"fcas{c}") for c in range(CH)]
        acc_q_fc = [pfc.tile([P, NCK], F32, name=f"fcaq{c}", tag=f"fcaq{c}") for c in range(CH)]

        psk_cm = tc.tile_pool(name="psk", bufs=3, space="PSUM")
        psk = psk_cm.__enter__()

        # K^T projection (o, tk) over full T, interleaved with fc chunks so
        # the ACT/DVE streams stay availability-ordered
        kproj = [(oc, tch) for oc in range(CH) for tch in range(T // 512)]
        for i, (oc, tch) in enumerate(kproj):
            if i % 2 == 0 and i // 2 < NCK * CH:
                kk, cc = divmod(i // 2, CH)
                stats_chunk(fcT, "fc", pfc, kk, cc, acc_s_fc, acc_q_fc,
                            round_to=fcr, round_cols=TH)
            pk = psk.tile([P, 512], F32, name=f"pk{oc}_{tch}", tag="pk")
            sl = slice(tch * 512, (tch + 1) * 512)
            nc.tensor.matmul(
                pk, wk[0][:, oc * P : (oc + 1) * P], fsr[0][:, sl],
                start=True, stop=False,
            )
            nc.tensor.matmul(
                pk, wk[1][:, oc * P : (oc + 1) * P], fsr[1][:, sl],
                start=False, stop=True,
            )
            if tch % 2 == 0:
                nc.scalar.activation(KTr[oc][:, sl], pk, AF.Identity, bias=bke[oc])
            else:
                nc.vector.tensor_scalar_add(KTr[oc][:, sl], pk, bke[oc])

        # fc stats -> folded Q weights + bias
        m_c, i_c = [], []
        for c in range(CH):
            m = pfc.tile([P, 1], F32, name=f"fcm{c}", tag=f"fcm{c}")
            nc.vector.reduce_sum(m, acc_s_fc[c], axis=mybir.AxisListType.X)
            nc.vector.tensor_scalar_mul(m, m, 1.0 / T)
            v = pfc.tile([P, 1], F32, name=f"fcv{c}", tag=f"fcv{c}")
            nc.vector.reduce_sum(v, acc_q_fc[c], axis=mybir.AxisListType.X)
            nc.vector.tensor_scalar_mul(v, v, 1.0 / T)
            msq = pfc.tile([P, 1], F32, name=f"fcmsq{c}", tag=f"fcmsq{c}")
            nc.vector.tensor_mul(msq, m, m)
            nc.vector.tensor_sub(v, v, msq)
            nc.scalar.activation(v, v, AF.Ln, bias=eps_t)
            nc.scalar.activation(v, v, AF.Exp, scale=-0.5)
            m_c.append(m)
            i_c.append(v)

        for c in range(CH):
            nc.vector.tensor_scalar_mul(wq[c], wf_sb[c], i_c[c])
        m_r = [pfc.tile([P, 2], F32R, name=f"fcmr{c}", tag=f"fcmr{c}") for c in range(CH)]
        for c in range(CH):
            nc.vector.tensor_copy(m_r[c], m_c[c].to_broadcast((P, 2)))
        for oc in range(CH):
            pb = psb.tile([P, 2], F32, name=f"pbq{oc}", tag="pbq")
            nc.tensor.matmul(pb, wq[0][:, oc * P : (oc + 1) * P], m_r[0], start=True, stop=False)
            nc.tensor.matmul(pb, wq[1][:, oc * P : (oc + 1) * P], m_r[1], start=False, stop=True)
            nc.vector.tensor_sub(bqe[oc], bq_sb[oc], pb[:, 0:1])

        # Q^T projection: core's own half is host-permuted to cols 0:TH
        for oc in range(CH):
            for tch in range(TH // 512):
                pq = psk.tile([P, 512], F32, name=f"pq{oc}_{tch}", tag="pk")
                sl = slice(tch * 512, (tch + 1) * 512)
                nc.tensor.matmul(
                    pq, wq[0][:, oc * P : (oc + 1) * P], fcr[0][:, sl],
                    start=True, stop=False,
                )
                nc.tensor.matmul(
                    pq, wq[1][:, oc * P : (oc + 1) * P], fcr[1][:, sl],
                    start=False, stop=True,
                )
                nc.scalar.activation(QTr[oc][:, sl], pq, AF.Identity, bias=bqe[oc])

        # fs/fc scratch + prologue PSUM no longer needed (LIFO order)
        psk_cm.__exit__(None, None, None)
        pfc_cm.__exit__(None, None, None)
        psb_cm.__exit__(None, None, None)
        pfs_cm.__exit__(None, None, None)
        pw_cm.__exit__(None, None, None)

        # ---------------- attention (fcs stats folded in) ------------------
        with tc.tile_pool(name="pfcs", bufs=1) as pfcs, tc.tile_pool(
            name="sts", bufs=5
        ) as sts, tc.tile_pool(name="epi", bufs=3) as epi, tc.tile_pool(
            name="psl", bufs=3, space="PSUM"
        ) as psl, tc.tile_pool(name="pmv", bufs=1, space="PSUM") as pmv, tc.tile_pool(
            name="pv2", bufs=1, space="PSUM"
        ) as pv2:
            acc_s_cs = [pfcs.tile([P, NCK], F32, name=f"csas{c}", tag=f"csas{c}") for c in range(CH)]
            acc_q_cs = [pfcs.tile([P, NCK], F32, name=f"csaq{c}", tag=f"csaq{c}") for c in range(CH)]
            # fcs chunk DMAs + per-quarter fcsh loads (interleaved in queue
            # order; the engine ops are deferred into the attention loop)
            fcs_ops = []
            for k in range(NCK):
                for c in range(CH):
                    ck = pfcs.tile([P, CK], F32, name=f"csck{c}_{k}", tag="csck", bufs=4)
                    nc.sync.dma_start(
                        out=ck, in_=fcsT[c * P : (c + 1) * P, k * CK : (k + 1) * CK]
                    )
                    fcs_ops.append((ck, k, c))
                nc.sync.dma_start(
                    out=nfcs[:, 4 * k : 4 * (k + 1), :],
                    in_=fcsh[4 * k * P : 4 * (k + 1) * P, :].rearrange(
                        "(n p) d -> p n d", p=P
                    ),
                )

            def emit_fcs_op(i):
                ck, k, c = fcs_ops[i]
                scr = pfcs.tile([P, CK], F32, name="csscr", tag="csscr", bufs=1)
                scr2 = pfcs.tile([P, CK], F32, name="csscr2", tag="csscr2", bufs=1)
                if (2 * k + c) % 2 == 0:
                    nc.scalar.activation(scr, ck, AF.Copy, accum_out=acc_s_cs[c][:, k : k + 1])
                    nc.vector.scalar_tensor_tensor(
                        scr2, ck, 0.0, ck, op0=OP.add, op1=OP.mult,
                        accum_out=acc_q_cs[c][:, k : k + 1],
                    )
                else:
                    nc.vector.tensor_scalar(
                        scr, ck, 0.0, 0.0, op0=OP.add, op1=OP.add,
                        accum_out=acc_s_cs[c][:, k : k + 1],
                    )
                    nc.scalar.activation(
                        scr2, ck, AF.Square, accum_out=acc_q_cs[c][:, k : k + 1]
                    )

            def emit_fcs_reduce():
                for c in range(CH):
                    m = pfcs.tile([P, 1], F32, name=f"csm{c}", tag=f"csm{c}")
                    nc.vector.reduce_sum(m, acc_s_cs[c], axis=mybir.AxisListType.X)
                    nc.vector.tensor_scalar_mul(m, m, 1.0 / T)
                    v = pfcs.tile([P, 1], F32, name=f"csv{c}", tag=f"csv{c}")
                    nc.vector.reduce_sum(v, acc_q_cs[c], axis=mybir.AxisListType.X)
                    nc.vector.tensor_scalar_mul(v, v, 1.0 / T)
                    msq = pfcs.tile([P, 1], F32, name=f"csmsq{c}", tag=f"csmsq{c}")
                    nc.vector.tensor_mul(msq, m, m)
                    nc.vector.tensor_sub(v, v, msq)
                    nc.scalar.activation(v, v, AF.Ln, bias=eps_t)
                    nc.scalar.activation(v, v, AF.Exp, scale=-0.5)
                    nc.gpsimd.dma_start(out=scm[0, c * P : (c + 1) * P], in_=m)
                    nc.gpsimd.dma_start(out=scm[1, c * P : (c + 1) * P], in_=v)
                nc.gpsimd.dma_start(out=m_bc, in_=_bcast_row(scm, 0, D))
                nc.gpsimd.dma_start(out=i_bc, in_=_bcast_row(scm, D, D))

            def emit_nfcs_norm(b):
                nc.gpsimd.tensor_sub(nfcs[:, b, :], nfcs[:, b, :], m_bc)
                nc.gpsimd.tensor_mul(nfcs[:, b, :], nfcs[:, b, :], i_bc)

            for q in range(NQ):  # tq chunks of 256
                qsl = slice(q * 256, (q + 1) * 256)
                mv = [pmv.tile([P, D + 2], F32, name=f"mv{q}_{i}", tag=f"mv{i}") for i in range(2)]
                v2 = [pv2.tile([P, D], F32, name=f"v2_{q}_{i}", tag=f"v2{i}") for i in range(2)]
                sts_tiles = [None] * (NB // 2)

                def emit_logits(jp, q=q, qsl=qsl, sts_tiles=sts_tiles):
                    pl = psl.tile([P, 512], F32, name=f"pl{q}_{jp}", tag="pl")
                    for h in range(2):
                        j = 2 * jp + h
                        osl = slice(h * 256, (h + 1) * 256)
                        nc.tensor.matmul(
                            pl[:, osl], KTr[0][:, j * P : (j + 1) * P], QTr[0][:, qsl],
                            start=True, stop=False,
                        )
                        nc.tensor.matmul(
                            pl[:, osl], KTr[1][:, j * P : (j + 1) * P], QTr[1][:, qsl],
                            start=False, stop=True,
                        )
                    st = sts.tile([P, 512], DT_AV, name="st", tag="st")
                    nc.scalar.activation(st, pl, AF.Exp, bias=negc0_t)
                    sts_tiles[jp] = st

                def emit_av(jp, q=q, mv=mv, v2=v2, sts_tiles=sts_tiles):
                    st = sts_tiles[jp]
                    for h in range(2):
                        j = 2 * jp + h
                        for b in range(2):
                            lhs = st[:, h * 256 + b * P : h * 256 + (b + 1) * P]
                            nc.tensor.matmul(
                                mv[b], lhs, Vr[:, j, :],
                                start=(j == 0), stop=(j == NB - 1),
                            )
                            nc.tensor.matmul(
                                v2[b], lhs, V2r[:, j, :],
                                start=(j == 0), stop=(j == NB - 1),
                            )

                emit_logits(0)
                for jp in range(1, NB // 2):
                    emit_logits(jp)
                    emit_av(jp - 1)
                    # fcs stats stream through q-chunk 0: one chunk op per
                    # odd jp matches the fcs DMA arrival cadence
                    if q == 0 and jp % 2 == 1:
                        emit_fcs_op(jp // 2)
                emit_av(NB // 2 - 1)
                # nfcs normalization must land before its consumer epilogue:
                # chunk q's epilogue reads nfcs blocks 2q, 2q+1
                if q == 0:
                    emit_fcs_reduce()
                    for b in range(4):
                        emit_nfcs_norm(b)
                elif q <= 6:
                    emit_nfcs_norm(2 * q + 2)
                    emit_nfcs_norm(2 * q + 3)

                for b in range(2):
                    qb = q * 2 + b
                    # evacuate PSUM right away so the next chunk's matmuls
                    # reuse the banks without waiting on the epilogue
                    mve = epi.tile([P, D + 2], F32, name="mve", tag="mve")
                    nc.vector.tensor_copy(mve, mv[b])
                    v2e = epi.tile([P, D], F32, name="v2e", tag="v2e")
                    nc.vector.tensor_copy(v2e, v2[b])
                    recip = epi.tile([P, 1], F32, name="recip", tag="recip")
                    nc.vector.reciprocal(recip, mve[:, D : D + 1])
                    Mt = epi.tile([P, D], F32, name="Mt", tag="Mt")
                    nc.vector.tensor_scalar_mul(Mt, mve[:, 0:D], recip)  # unbiased M
                    Msq = epi.tile([P, D], F32, name="Msq", tag="Msq")
                    nc.vector.tensor_mul(Msq, Mt, Mt)
                    # Var -> v2e (in place), clamp, S = Exp(0.5*Ln(Var))
                    nc.vector.scalar_tensor_tensor(
                        v2e, v2e, recip, Msq, op0=OP.mult, op1=OP.subtract
                    )
                    nc.vector.tensor_scalar_max(v2e, v2e, EPS_VAR)
                    nc.scalar.activation(Msq, v2e, AF.Ln)
                    nc.scalar.activation(Msq, Msq, AF.Exp, scale=0.5)
                    # out = S*nfcs + M + bh (final two on GPSIMD: SBUF-only ops)
                    Mb = epi.tile([P, D], F32, name="Mb", tag="Mb")
                    nc.vector.tensor_add(Mb, Mt, bv_bc)
                    nc.gpsimd.tensor_mul(Msq, Msq, nfcs[:, qb, :])
                    nc.gpsimd.tensor_add(Msq, Msq, Mb)
                    nc.sync.dma_start(out=out_e[qb * P : (qb + 1) * P, :], in_=Msq)

        persist_cm.__exit__(None, None, None)

    nc.compile()
    return nc


_CACHE = {}


def _get_nc():
    if "nc" not in _CACHE:
        _CACHE["nc"] = build_nc()
    return _CACHE["nc"]


def kernel(**inputs):
    fc = np.ascontiguousarray(np.asarray(inputs["fc"], dtype=np.float32))
    fs = np.ascontiguousarray(np.asarray(inputs["fs"], dtype=np.float32))
    fcs = np.ascontiguousarray(np.asarray(inputs["fcs"], dtype=np.float32))
    Wf = np.asarray(inputs["Wf"], dtype=np.float32)
    bf = np.asarray(inputs["bf"], dtype=np.float32)
    Wg = np.asarray(inputs["Wg"], dtype=np.float32)
    bg = np.asarray(inputs["bg"], dtype=np.float32)
    Wh = np.asarray(inputs["Wh"], dtype=np.float32)
    bh = np.asarray(inputs["bh"], dtype=np.float32)

    wfT = np.ascontiguousarray(Wf.T)
    wgT = np.ascontiguousarray(Wg.T)
    whT = np.ascontiguousarray(Wh.T)
    bq = np.ascontiguousarray(bf.reshape(D, 1))
    bk = np.ascontiguousarray(bg.reshape(D, 1))

    in_maps = []
    for core in range(8):
        s, h = divmod(core, 2)
        fcT_s = fc[s].T  # (D, T)
        if h == 0:
            fcT_perm = np.ascontiguousarray(fcT_s)
        else:
            fcT_perm = np.ascontiguousarray(
                np.concatenate([fcT_s[:, TH:], fcT_s[:, :TH]], axis=1)
            )
        in_maps.append(
            {
                "fcT": fcT_perm,
                "fsT": np.ascontiguousarray(fs[s].T),
                "fcsT": np.ascontiguousarray(fcs[s].T),
                "fcsh": np.ascontiguousarray(fcs[s, h * TH : (h + 1) * TH, :]),
                "wfT": wfT,
                "wgT": wgT,
                "whT": whT,
                "bq": bq,
                "bk": bk,
                "bv": bh,
            }
        )

    nc = _get_nc()
    res = run_bass_kernel_spmd(
        nc, in_maps, core_ids=list(range(8)), trace=TRACE, **TRACE_KW
    )
    if TRACE:
        _CACHE["last_result"] = res

    out = np.empty((4, T, D), np.float32)
    for core in range(8):
        s, h = divmod(core, 2)
        out[s, h * TH : (h + 1) * TH, :] = res.results[core]["out"]
    return out


# revision 19
# speedup vs baseline: 1.0935x; 1.0935x over previous
"""AdaAttN attention kernel for 8 TRN2 NeuronCores (v2).

Problem: nn_AdaAttN_29076928593982
  fc, fs, fcs: (4, 4096, 256) f32; Wf/Wg/Wh (256,256); bf/bg/bh (256,)
  Q = Wf@inorm(fc_t)+bf; K = Wg@inorm(fs_t)+bg; V = Wh@fs_t+bh
  A = softmax(Q K); M = A V; Var = A V^2 - M^2; S = sqrt(max(Var,1e-6))
  out = S * inorm(fcs_t) + M   (all in (b, t, d))

Sharding: data-parallel over (sample, query-half): core i -> sample i//2,
query rows [ (i%2)*2048, +2048 ). K/V replicated per sample. No collectives.

v2 changes over the phase-serial baseline (302us):
  - V projection needs NO instance norm, so it streams chunk-by-chunk
    against the fs DMA: the PE starts working ~2us in and never idles
    long enough for the HAM clock-gate to re-throttle.
  - One fused emission order (PE executes its queue in order): V proj ->
    K bias/proj -> Q bias/proj -> attention; fc/fcs stats run on ACT/DVE
    interleaved so no engine stream blocks on a DMA that lands later.
  - softmax tile (st), V and V^2 are bf16: bf16 stationaries get FWL
    (~4x faster LDWEIGHTS), un-saturating the PE weight path that f32r
    (no FWL) saturates; A-quantization at 0.4%/weight is benign since
    per-row scale errors cancel in M = (A@V)/(A@1).
  - sqrt via Exp(0.5*Ln(x)): Exp and Sqrt never share an ACT function
    table (22 table loads = 28us in the baseline trace); Ln+Exp co-reside
    in natural_log_exp_and_others so the table loads once.
  - logits stay f32r (bf16 Q/K would put ~0.07 abs noise on logits which
    the softmax exponentiates to ~7% weight errors).
"""
import sys

sys.path.insert(0, "/opt/trn_rl_repo")

import numpy as np

import concourse.bass as bass
import concourse.tile as tile
from concourse import bacc
from concourse import mybir
from concourse.bass_utils import run_bass_kernel_spmd

F32 = mybir.dt.float32
F32R = mybir.dt.float32r
BF16 = mybir.dt.bfloat16
AF = mybir.ActivationFunctionType
OP = mybir.AluOpType

P = 128          # partitions
D = 256          # feature dim
T = 4096         # tokens per sample
TH = 2048        # query tokens per core
CH = 2           # channel chunks (D // P)
NB = T // P      # tk chunks (32)
NQ = TH // 256   # tq chunks of 256 (8)
C0 = 110.0       # global softmax shift
EPS_IN = 1e-5
EPS_VAR = 1e-6
CK = 1024        # stats DMA chunk width
NCK = T // CK

# Attention matmul dtypes. The BIR verifier requires f32/f32r operands to
# pair with the SAME dtype, so bf16 stationaries (which would get fast FWL
# weight loads) force bf16 V too -- and bf16 V costs ~1.7e-2 rel err through
# the Var = E[v^2] - M^2 cancellation (measured). All-f32r keeps ~4e-3.
DT_ST = F32R
DT_V = F32R

TRACE = False    # test.py sets this to get exec_time_ns
TRACE_KW = {}

ACT_TABLE = "natural_log_exp_and_others"  # covers Copy/Exp/Identity/Ln/Square


class _Bacc(bacc.Bacc):
    """Bacc that pins all activations to one ACT function table.

    The stock pass assigns each activation the FIRST table containing its
    function, so Exp->exp_and_others but Ln->natural_log_exp_and_others:
    interleaved Ln/Exp then thrash 1.28us ACT_TABLE_LOADs on the critical
    exp stream (41 loads = 53us measured). Emptying every other table's
    function set (list positions kept, so set ids stay canonical) makes
    every function resolve to the one table, loaded once.
    """

    def insert_act_table_loads(self):
        import bass_rust as _bass_rust
        from concourse.hw_specs import get_activation_tables

        has_activation = any(
            isinstance(i, mybir.InstActivation)
            for b in self.main_func.blocks
            for i in b.instructions
        )
        if not has_activation:
            return
        tables = [
            (name, (fns if name == ACT_TABLE else set()))
            for name, fns in get_activation_tables(self.m.arch).items()
        ]
        _bass_rust.insert_act_table_loads(self, tables)


def _bcast_row(handle, offset, n):
    """AP reading a DRAM row of n elements broadcast across 128 partitions."""
    return bass.AP(tensor=handle, offset=offset, ap=[[0, P], [1, n]])


def build_nc():
    nc = _Bacc()

    fcT = nc.declare_dram_parameter("fcT", [D, T], F32, isOutput=False)
    fsT = nc.declare_dram_parameter("fsT", [D, T], F32, isOutput=False)
    fcsT = nc.declare_dram_parameter("fcsT", [D, T], F32, isOutput=False)
    fcsh = nc.declare_dram_parameter("fcsh", [TH, D], F32, isOutput=False)
    wfT = nc.declare_dram_parameter("wfT", [D, D], F32, isOutput=False)
    wgT = nc.declare_dram_parameter("wgT", [D, D], F32, isOutput=False)
    whT = nc.declare_dram_parameter("whT", [D, D], F32, isOutput=False)
    bq_e = nc.declare_dram_parameter("bq", [D, 1], F32, isOutput=False)
    bk_e = nc.declare_dram_parameter("bk", [D, 1], F32, isOutput=False)
    bv_e = nc.declare_dram_parameter("bv", [D], F32, isOutput=False)
    out_e = nc.declare_dram_parameter("out", [TH, D], F32, isOutput=True)

    scm = nc.dram_tensor("scm", [2, D], F32)  # fcs stats roundtrip scratch

    with tile.TileContext(nc) as tc:
        persist_cm = tc.tile_pool(name="persist", bufs=1)
        pp = persist_cm.__enter__()

        QTr = [pp.tile([P, TH], F32R, name=f"qtr{c}", tag=f"qtr{c}") for c in range(CH)]
        KTr = [pp.tile([P, T], F32R, name=f"ktr{c}", tag=f"ktr{c}") for c in range(CH)]
        Vr = pp.tile([P, NB, D + 2], DT_V, name="vr", tag="vr")  # [V | ones | pad]
        V2r = pp.tile([P, NB, D], DT_V, name="v2r", tag="v2r")
        bqe = [pp.tile([P, 1], F32, name=f"bqe{c}", tag=f"bqe{c}") for c in range(CH)]
        bke = [pp.tile([P, 1], F32, name=f"bke{c}", tag=f"bke{c}") for c in range(CH)]
        bv_bc = pp.tile([P, D], F32, name="bvbc", tag="bvbc")
        eps_t = pp.tile([P, 1], F32, name="epsin", tag="epsin")
        negc0_t = pp.tile([P, 1], F32, name="negc0", tag="negc0")

        # weight staging + folded copies, live for the whole prologue
        pw_cm = tc.tile_pool(name="pw", bufs=1)
        pw = pw_cm.__enter__()
        wf_sb = [pw.tile([P, D], F32, name=f"wf{c}", tag=f"wf{c}") for c in range(CH)]
        wg_sb = [pw.tile([P, D], F32, name=f"wg{c}", tag=f"wg{c}") for c in range(CH)]
        wh_sb = [pw.tile([P, D], F32, name=f"wh{c}", tag=f"wh{c}") for c in range(CH)]
        bq_sb = [pw.tile([P, 1], F32, name=f"bqs{c}", tag=f"bqs{c}") for c in range(CH)]
        bk_sb = [pw.tile([P, 1], F32, name=f"bks{c}", tag=f"bks{c}") for c in range(CH)]
        wq = [pw.tile([P, D], F32R, name=f"wq{c}", tag=f"wq{c}") for c in range(CH)]
        wk = [pw.tile([P, D], F32R, name=f"wk{c}", tag=f"wk{c}") for c in range(CH)]
        wv = [pw.tile([P, D], F32R, name=f"wv{c}", tag=f"wv{c}") for c in range(CH)]

        # weight DMAs go on the gpsimd queue so the sync queue starts
        # streaming fs at t=0 (ten small latency-bound transfers at the
        # head of the sync queue delayed the first fs chunk ~5us)
        for c in range(CH):
            nc.gpsimd.dma_start(out=wh_sb[c], in_=whT[c * P : (c + 1) * P, :])
            nc.gpsimd.dma_start(out=wg_sb[c], in_=wgT[c * P : (c + 1) * P, :])
            nc.gpsimd.dma_start(out=wf_sb[c], in_=wfT[c * P : (c + 1) * P, :])
            nc.gpsimd.dma_start(out=bq_sb[c], in_=bq_e[c * P : (c + 1) * P, :])
            nc.gpsimd.dma_start(out=bk_sb[c], in_=bk_e[c * P : (c + 1) * P, :])
        nc.gpsimd.dma_start(out=bv_bc, in_=_bcast_row(bv_e, 0, D))

        nc.vector.memset(eps_t, EPS_IN)
        nc.vector.memset(negc0_t, -C0)
        ones_f32 = pw.tile([P, NB * 2], F32, name="ones32", tag="ones32")
        nc.vector.memset(ones_f32, 1.0)
        nc.vector.tensor_copy(
            Vr[:, :, D : D + 2], ones_f32.rearrange("p (n two) -> p n two", two=2)
        )
        for c in range(CH):
            nc.vector.tensor_copy(wv[c], wh_sb[c])

        def stats_chunk(x_ext, name, ring, k, c, acc_s, acc_q, round_to=None,
                        round_cols=0, bufs=4):
            """Load chunk (c,k) of a (D,T) DRAM tensor, accumulate sum and
            sumsq (ACT/DVE alternating), writing the f32r rounded copy."""
            ck = ring.tile([P, CK], F32, name=f"{name}ck{c}_{k}", tag=f"{name}ck", bufs=bufs)
            nc.sync.dma_start(
                out=ck, in_=x_ext[c * P : (c + 1) * P, k * CK : (k + 1) * CK]
            )
            scr = pw.tile([P, CK], F32, name=f"{name}scr", tag="scr", bufs=1)
            if round_to is not None and (k + 1) * CK <= round_cols:
                dst = round_to[c][:, k * CK : (k + 1) * CK]
            else:
                dst = scr
            scr2 = pw.tile([P, CK], F32, name=f"{name}scr2", tag="scr2", bufs=1)
            if (2 * k + c) % 2 == 0:
                nc.scalar.activation(dst, ck, AF.Copy, accum_out=acc_s[c][:, k : k + 1])
                nc.vector.scalar_tensor_tensor(
                    scr2, ck, 0.0, ck, op0=OP.add, op1=OP.mult,
                    accum_out=acc_q[c][:, k : k + 1],
                )
            else:
                nc.vector.tensor_scalar(
                    dst, ck, 0.0, 0.0, op0=OP.add, op1=OP.add,
                    accum_out=acc_s[c][:, k : k + 1],
                )
                nc.scalar.activation(
                    scr2, ck, AF.Square, accum_out=acc_q[c][:, k : k + 1]
                )

        # ---------------- fs phase: V proj streamed against the DMA -------
        pfs_cm = tc.tile_pool(name="pfs", bufs=1)
        pfs = pfs_cm.__enter__()
        fsr = [pfs.tile([P, T], F32R, name=f"fsr{c}", tag=f"fsr{c}") for c in range(CH)]
        acc_s_fs = [pfs.tile([P, NCK], F32, name=f"fsas{c}", tag=f"fsas{c}") for c in range(CH)]
        acc_q_fs = [pfs.tile([P, NCK], F32, name=f"fsaq{c}", tag=f"fsaq{c}") for c in range(CH)]

        psv_cm = tc.tile_pool(name="psv", bufs=3, space="PSUM")
        psv = psv_cm.__enter__()

        for k in range(NCK):
            for c in range(CH):
                stats_chunk(fsT, "fs", pfs, k, c, acc_s_fs, acc_q_fs,
                            round_to=fsr, round_cols=T)
            # V proj for the 8 token blocks this chunk completes
            # (V = Wh @ fs + bh has no instance norm: no stats dependency)
            for tb in range(8 * k, 8 * (k + 1)):
                pv = psv.tile([P, D], F32, name=f"pv{tb}", tag="pv")
                sl = slice(tb * P, (tb + 1) * P)
                nc.tensor.matmul(pv, fsr[0][:, sl], wv[0], start=True, stop=False)
                nc.tensor.matmul(pv, fsr[1][:, sl], wv[1], start=False, stop=True)
                if tb % 2 == 0:
                    nc.scalar.activation(Vr[:, tb, 0:D], pv, AF.Copy)
                else:
                    nc.vector.tensor_copy(Vr[:, tb, 0:D], pv)
                nc.vector.tensor_mul(V2r[:, tb, :], Vr[:, tb, 0:D], Vr[:, tb, 0:D])

        psv_cm.__exit__(None, None, None)

        # fs stats: mean + inv_std (rsqrt via Ln/Exp, no table swap)
        m_s, i_s = [], []
        for c in range(CH):
            m = pfs.tile([P, 1], F32, name=f"fsm{c}", tag=f"fsm{c}")
            nc.vector.reduce_sum(m, acc_s_fs[c], axis=mybir.AxisListType.X)
            nc.vector.tensor_scalar_mul(m, m, 1.0 / T)
            v = pfs.tile([P, 1], F32, name=f"fsv{c}", tag=f"fsv{c}")
            nc.vector.reduce_sum(v, acc_q_fs[c], axis=mybir.AxisListType.X)
            nc.vector.tensor_scalar_mul(v, v, 1.0 / T)
            msq = pfs.tile([P, 1], F32, name=f"fsmsq{c}", tag=f"fsmsq{c}")
            nc.vector.tensor_mul(msq, m, m)
            nc.vector.tensor_sub(v, v, msq)
            nc.scalar.activation(v, v, AF.Ln, bias=eps_t)
            nc.scalar.activation(v, v, AF.Exp, scale=-0.5)
            m_s.append(m)
            i_s.append(v)

        psb_cm = tc.tile_pool(name="psb", bufs=2, space="PSUM")
        psb = psb_cm.__enter__()

        for c in range(CH):
            nc.vector.tensor_scalar_mul(wk[c], wg_sb[c], i_s[c])
        m_sr = [pfs.tile([P, 2], F32R, name=f"fsmr{c}", tag=f"fsmr{c}") for c in range(CH)]
        for c in range(CH):
            nc.vector.tensor_copy(m_sr[c], m_s[c].to_broadcast((P, 2)))
        for oc in range(CH):
            pb = psb.tile([P, 2], F32, name=f"pbk{oc}", tag="pbk")
            nc.tensor.matmul(pb, wk[0][:, oc * P : (oc + 1) * P], m_sr[0], start=True, stop=False)
            nc.tensor.matmul(pb, wk[1][:, oc * P : (oc + 1) * P], m_sr[1], start=False, stop=True)
            nc.vector.tensor_sub(bke[oc], bk_sb[oc], pb[:, 0:1])

        # ---------------- fc stats stream + K^T projection interleaved ----
        pfc_cm = tc.tile_pool(name="pfc", bufs=1)
        pfc = pfc_cm.__enter__()
        fcr = [pfc.tile([P, TH], F32R, name=f"fcr{c}", tag=f"fcr{c}") for c in range(CH)]
        acc_s_fc = [pfc.tile([P, NCK], F32, name=f"fcas{c}", tag=f"fcas{c}") for c in range(CH)]
        acc_q_fc = [pfc.tile([P, NCK], F32, name=f"fcaq{c}", tag=f"fcaq{c}") for c in range(CH)]

        psk_cm = tc.tile_pool(name="psk", bufs=3, space="PSUM")
        psk = psk_cm.__enter__()

        # K^T projection (o, tk) over full T, interleaved with fc chunks so
        # the ACT/DVE streams stay availability-ordered
        kproj = [(oc, tch) for oc in range(CH) for tch in range(T // 512)]
        for i, (oc, tch) in enumerate(kproj):
            if i % 2 == 0 and i // 2 < NCK * CH:
                kk, cc = divmod(i // 2, CH)
                stats_chunk(fcT, "fc", pfc, kk, cc, acc_s_fc, acc_q_fc,
                            round_to=fcr, round_cols=TH, bufs=2)
            pk = psk.tile([P, 512], F32, name=f"pk{oc}_{tch}", tag="pk")
            sl = slice(tch * 512, (tch + 1) * 512)
            nc.tensor.matmul(
                pk, wk[0][:, oc * P : (oc + 1) * P], fsr[0][:, sl],
                start=True, stop=False,
            )
            nc.tensor.matmul(
                pk, wk[1][:, oc * P : (oc + 1) * P], fsr[1][:, sl],
                start=False, stop=True,
            )
            if tch % 2 == 0:
                nc.scalar.activation(KTr[oc][:, sl], pk, AF.Identity, bias=bke[oc])
            else:
                nc.vector.tensor_scalar_add(KTr[oc][:, sl], pk, bke[oc])

        # fc stats -> folded Q weights + bias
        m_c, i_c = [], []
        for c in range(CH):
            m = pfc.tile([P, 1], F32, name=f"fcm{c}", tag=f"fcm{c}")
            nc.vector.reduce_sum(m, acc_s_fc[c], axis=mybir.AxisListType.X)
            nc.vector.tensor_scalar_mul(m, m, 1.0 / T)
            v = pfc.tile([P, 1], F32, name=f"fcv{c}", tag=f"fcv{c}")
            nc.vector.reduce_sum(v, acc_q_fc[c], axis=mybir.AxisListType.X)
            nc.vector.tensor_scalar_mul(v, v, 1.0 / T)
            msq = pfc.tile([P, 1], F32, name=f"fcmsq{c}", tag=f"fcmsq{c}")
            nc.vector.tensor_mul(msq, m, m)
            nc.vector.tensor_sub(v, v, msq)
            nc.scalar.activation(v, v, AF.Ln, bias=eps_t)
            nc.scalar.activation(v, v, AF.Exp, scale=-0.5)
            m_c.append(m)
            i_c.append(v)

        for c in range(CH):
            nc.vector.tensor_scalar_mul(wq[c], wf_sb[c], i_c[c])
        m_r = [pfc.tile([P, 2], F32R, name=f"fcmr{c}", tag=f"fcmr{c}") for c in range(CH)]
        for c in range(CH):
            nc.vector.tensor_copy(m_r[c], m_c[c].to_broadcast((P, 2)))
        for oc in range(CH):
            pb = psb.tile([P, 2], F32, name=f"pbq{oc}", tag="pbq")
            nc.tensor.matmul(pb, wq[0][:, oc * P : (oc + 1) * P], m_r[0], start=True, stop=False)
            nc.tensor.matmul(pb, wq[1][:, oc * P : (oc + 1) * P], m_r[1], start=False, stop=True)
            nc.vector.tensor_sub(bqe[oc], bq_sb[oc], pb[:, 0:1])

        # Q^T projection: core's own half is host-permuted to cols 0:TH
        for oc in range(CH):
            for tch in range(TH // 512):
                pq = psk.tile([P, 512], F32, name=f"pq{oc}_{tch}", tag="pk")
                sl = slice(tch * 512, (tch + 1) * 512)
                nc.tensor.matmul(
                    pq, wq[0][:, oc * P : (oc + 1) * P], fcr[0][:, sl],
                    start=True, stop=False,
                )
                nc.tensor.matmul(
                    pq, wq[1][:, oc * P : (oc + 1) * P], fcr[1][:, sl],
                    start=False, stop=True,
                )
                nc.scalar.activation(QTr[oc][:, sl], pq, AF.Identity, bias=bqe[oc])

        # fs/fc scratch + prologue PSUM no longer needed (LIFO order)
        psk_cm.__exit__(None, None, None)
        pfc_cm.__exit__(None, None, None)
        psb_cm.__exit__(None, None, None)
        pfs_cm.__exit__(None, None, None)
        pw_cm.__exit__(None, None, None)

        # ---------------- attention (fcs stats folded in) ------------------
        with tc.tile_pool(name="pfcs", bufs=1) as pfcs, tc.tile_pool(
            name="sts", bufs=5
        ) as sts, tc.tile_pool(name="epi", bufs=3) as epi, tc.tile_pool(
            name="psl", bufs=3, space="PSUM"
        ) as psl, tc.tile_pool(name="pmv", bufs=1, space="PSUM") as pmv, tc.tile_pool(
            name="pv2", bufs=1, space="PSUM"
        ) as pv2:
            nfcs = pfcs.tile([P, TH // P, D], F32, name="nfcs", tag="nfcs")
            m_bc = pfcs.tile([P, D], F32, name="mbc", tag="mbc")
            i_bc = pfcs.tile([P, D], F32, name="ibc", tag="ibc")
            acc_s_cs = [pfcs.tile([P, NCK], F32, name=f"csas{c}", tag=f"csas{c}") for c in range(CH)]
            acc_q_cs = [pfcs.tile([P, NCK], F32, name=f"csaq{c}", tag=f"csaq{c}") for c in range(CH)]
            # fcs chunk DMAs + per-quarter fcsh loads (interleaved in queue
            # order; the engine ops are deferred into the attention loop)
            fcs_ops = []
            for k in range(NCK):
                for c in range(CH):
                    ck = pfcs.tile([P, CK], F32, name=f"csck{c}_{k}", tag="csck", bufs=4)
                    nc.sync.dma_start(
                        out=ck, in_=fcsT[c * P : (c + 1) * P, k * CK : (k + 1) * CK]
                    )
                    fcs_ops.append((ck, k, c))
                nc.sync.dma_start(
                    out=nfcs[:, 4 * k : 4 * (k + 1), :],
                    in_=fcsh[4 * k * P : 4 * (k + 1) * P, :].rearrange(
                        "(n p) d -> p n d", p=P
                    ),
                )

            def emit_fcs_op(i):
                ck, k, c = fcs_ops[i]
                scr = pfcs.tile([P, CK], F32, name="csscr", tag="csscr", bufs=1)
                scr2 = pfcs.tile([P, CK], F32, name="csscr2", tag="csscr2", bufs=1)
                if (2 * k + c) % 2 == 0:
                    nc.scalar.activation(scr, ck, AF.Copy, accum_out=acc_s_cs[c][:, k : k + 1])
                    nc.vector.scalar_tensor_tensor(
                        scr2, ck, 0.0, ck, op0=OP.add, op1=OP.mult,
                        accum_out=acc_q_cs[c][:, k : k + 1],
                    )
                else:
                    nc.vector.tensor_scalar(
                        scr, ck, 0.0, 0.0, op0=OP.add, op1=OP.add,
                        accum_out=acc_s_cs[c][:, k : k + 1],
                    )
                    nc.scalar.activation(
                        scr2, ck, AF.Square, accum_out=acc_q_cs[c][:, k : k + 1]
                    )

            def emit_fcs_reduce():
                for c in range(CH):
                    m = pfcs.tile([P, 1], F32, name=f"csm{c}", tag=f"csm{c}")
                    nc.vector.reduce_sum(m, acc_s_cs[c], axis=mybir.AxisListType.X)
                    nc.vector.tensor_scalar_mul(m, m, 1.0 / T)
                    v = pfcs.tile([P, 1], F32, name=f"csv{c}", tag=f"csv{c}")
                    nc.vector.reduce_sum(v, acc_q_cs[c], axis=mybir.AxisListType.X)
                    nc.vector.tensor_scalar_mul(v, v, 1.0 / T)
                    msq = pfcs.tile([P, 1], F32, name=f"csmsq{c}", tag=f"csmsq{c}")
                    nc.vector.tensor_mul(msq, m, m)
                    nc.vector.tensor_sub(v, v, msq)
                    nc.scalar.activation(v, v, AF.Ln, bias=eps_t)
                    nc.scalar.activation(v, v, AF.Exp, scale=-0.5)
                    nc.gpsimd.dma_start(out=scm[0, c * P : (c + 1) * P], in_=m)
                    nc.gpsimd.dma_start(out=scm[1, c * P : (c + 1) * P], in_=v)
                nc.gpsimd.dma_start(out=m_bc, in_=_bcast_row(scm, 0, D))
                nc.gpsimd.dma_start(out=i_bc, in_=_bcast_row(scm, D, D))

            def emit_nfcs_norm(b):
                nc.gpsimd.tensor_sub(nfcs[:, b, :], nfcs[:, b, :], m_bc)
                nc.gpsimd.tensor_mul(nfcs[:, b, :], nfcs[:, b, :], i_bc)

            for q in range(NQ):  # tq chunks of 256
                qsl = slice(q * 256, (q + 1) * 256)
                mv = [pmv.tile([P, D + 2], F32, name=f"mv{q}_{i}", tag=f"mv{i}") for i in range(2)]
                v2 = [pv2.tile([P, D], F32, name=f"v2_{q}_{i}", tag=f"v2{i}") for i in range(2)]
                sts_tiles = [None] * (NB // 2)

                def emit_logits(jp, q=q, qsl=qsl, sts_tiles=sts_tiles):
                    pl = psl.tile([P, 512], F32, name=f"pl{q}_{jp}", tag="pl")
                    for h in range(2):
                        j = 2 * jp + h
                        osl = slice(h * 256, (h + 1) * 256)
                        nc.tensor.matmul(
                            pl[:, osl], KTr[0][:, j * P : (j + 1) * P], QTr[0][:, qsl],
                            start=True, stop=False,
                        )
                        nc.tensor.matmul(
                            pl[:, osl], KTr[1][:, j * P : (j + 1) * P], QTr[1][:, qsl],
                            start=False, stop=True,
                        )
                    st = sts.tile([P, 512], DT_ST, name="st", tag="st")
                    nc.scalar.activation(st, pl, AF.Exp, bias=negc0_t)
                    sts_tiles[jp] = st

                def emit_av(jp, q=q, mv=mv, v2=v2, sts_tiles=sts_tiles):
                    st = sts_tiles[jp]
                    for h in range(2):
                        j = 2 * jp + h
                        for b in range(2):
                            lhs = st[:, h * 256 + b * P : h * 256 + (b + 1) * P]
                            nc.tensor.matmul(
                                mv[b], lhs, Vr[:, j, :],
                                start=(j == 0), stop=(j == NB - 1),
                            )
                            nc.tensor.matmul(
                                v2[b], lhs, V2r[:, j, :],
                                start=(j == 0), stop=(j == NB - 1),
                            )

                emit_logits(0)
                for jp in range(1, NB // 2):
                    emit_logits(jp)
                    emit_av(jp - 1)
                    # fcs stats stream through q-chunk 0, front-loaded so the
                    # reduce -> DRAM-broadcast roundtrip -> nfcs norm chain
                    # completes before this chunk's epilogue needs nfcs
                    if q == 0:
                        if jp <= 8:
                            emit_fcs_op(jp - 1)
                        elif jp == 9:
                            emit_fcs_reduce()
                        elif jp == 12:
                            for b in range(4):
                                emit_nfcs_norm(b)
                emit_av(NB // 2 - 1)
                # chunk q's epilogue reads nfcs blocks 2q, 2q+1
                if 1 <= q <= 6:
                    emit_nfcs_norm(2 * q + 2)
                    emit_nfcs_norm(2 * q + 3)

                for b in range(2):
                    qb = q * 2 + b
                    # evacuate PSUM right away so the next chunk's matmuls
                    # reuse the banks without waiting on the epilogue
                    mve = epi.tile([P, D + 2], F32, name="mve", tag="mve")
                    nc.vector.tensor_copy(mve, mv[b])
                    v2e = epi.tile([P, D], F32, name="v2e", tag="v2e")
                    nc.vector.tensor_copy(v2e, v2[b])
                    recip = epi.tile([P, 1], F32, name="recip", tag="recip")
                    nc.vector.reciprocal(recip, mve[:, D : D + 1])
                    Mt = epi.tile([P, D], F32, name="Mt", tag="Mt")
                    nc.vector.tensor_scalar_mul(Mt, mve[:, 0:D], recip)  # unbiased M
                    Msq = epi.tile([P, D], F32, name="Msq", tag="Msq")
                    nc.vector.tensor_mul(Msq, Mt, Mt)
                    # Var -> v2e (in place), clamp, S = Exp(0.5*Ln(Var))
                    nc.vector.scalar_tensor_tensor(
                        v2e, v2e, recip, Msq, op0=OP.mult, op1=OP.subtract
                    )
                    nc.vector.tensor_scalar_max(v2e, v2e, EPS_VAR)
                    nc.scalar.activation(Msq, v2e, AF.Ln)
                    nc.scalar.activation(Msq, Msq, AF.Exp, scale=0.5)
                    # out = S*nfcs + M + bh (final two on GPSIMD: SBUF-only ops)
                    Mb = epi.tile([P, D], F32, name="Mb", tag="Mb")
                    nc.vector.tensor_add(Mb, Mt, bv_bc)
                    nc.gpsimd.tensor_mul(Msq, Msq, nfcs[:, qb, :])
                    nc.gpsimd.tensor_add(Msq, Msq, Mb)
                    nc.sync.dma_start(out=out_e[qb * P : (qb + 1) * P, :], in_=Msq)

        persist_cm.__exit__(None, None, None)

    nc.compile()
    return nc


_CACHE = {}


def _get_nc():
    if "nc" not in _CACHE:
        _CACHE["nc"] = build_nc()
    return _CACHE["nc"]


def kernel(**inputs):
    fc = np.ascontiguousarray(np.asarray(inputs["fc"], dtype=np.float32))
    fs = np.ascontiguousarray(np.asarray(inputs["fs"], dtype=np.float32))
    fcs = np.ascontiguousarray(np.asarray(inputs["fcs"], dtype=np.float32))
    Wf = np.asarray(inputs["Wf"], dtype=np.float32)
    bf = np.asarray(inputs["bf"], dtype=np.float32)
    Wg = np.asarray(inputs["Wg"], dtype=np.float32)
    bg = np.asarray(inputs["bg"], dtype=np.float32)
    Wh = np.asarray(inputs["Wh"], dtype=np.float32)
    bh = np.asarray(inputs["bh"], dtype=np.float32)

    wfT = np.ascontiguousarray(Wf.T)
    wgT = np.ascontiguousarray(Wg.T)
    whT = np.ascontiguousarray(Wh.T)
    bq = np.ascontiguousarray(bf.reshape(D, 1))
    bk = np.ascontiguousarray(bg.reshape(D, 1))

    in_maps = []
    for core in range(8):
        s, h = divmod(core, 2)
        fcT_s = fc[s].T  # (D, T)
        if h == 0:
            fcT_perm = np.ascontiguousarray(fcT_s)
        else:
            fcT_perm = np.ascontiguousarray(
                np.concatenate([fcT_s[:, TH:], fcT_s[:, :TH]], axis=1)
            )
        in_maps.append(
            {
                "fcT": fcT_perm,
                "fsT": np.ascontiguousarray(fs[s].T),
                "fcsT": np.ascontiguousarray(fcs[s].T),
                "fcsh": np.ascontiguousarray(fcs[s, h * TH : (h + 1) * TH, :]),
                "wfT": wfT,
                "wgT": wgT,
                "whT": whT,
                "bq": bq,
                "bk": bk,
                "bv": bh,
            }
        )

    nc = _get_nc()
    res = run_bass_kernel_spmd(
        nc, in_maps, core_ids=list(range(8)), trace=TRACE, **TRACE_KW
    )
    if TRACE:
        _CACHE["last_result"] = res

    out = np.empty((4, T, D), np.float32)
    for core in range(8):
        s, h = divmod(core, 2)
        out[s, h * TH : (h + 1) * TH, :] = res.results[core]["out"]
    return out
